# revision 1
# baseline (speedup 1.0000x reference)
"""Trainium2 Bass kernel for nn_Decoder (MLP -> inverse token embedding ->
overlap-add -> channel-merge conv), data-parallel over batch on 8 NeuronCores.

Self-contained: hardcodes shapes; host-side numpy folds everything after the
first Linear+ReLU into per-channel fused matrices G (W2 -> Winv -> overlap-add
normalization -> 3-tap channel conv), so the device pipeline is:

    x[tok,E] --PE transpose--> xT[E,tok] --matmul W1T--> h[Hc,tok] in PSUM
    --ACT/DVE relu+bias--> hT in SBUF --matmul G (accum over c,Hc)--> v[66,tok]
    --PE transpose--> vT[b,66] --strided DVE adds (overlap-add)--> y[b,1056]

Sharding: batch 1024 -> 8 cores x 128.
"""

import numpy as np

import concourse.bacc as bacc
import concourse.mybir as mybir
from concourse.bass_utils import run_bass_kernel_spmd
from concourse.tile import TileContext

# problem shapes (hardcoded per contract)
B, C, T, E, H = 1024, 8, 32, 128, 256
SEG_LEN, SIG_LEN, NUM_SEG, STEP = 64, 1056, 32, 32
N_CORES = 8
BL = B // N_CORES          # local batch per core = 128
HC = H // 128              # H chunks = 2
TC = 8                     # t-chunks
TL = T // TC               # t per chunk = 4
FD = mybir.dt.float32
FR = mybir.dt.float32r   # fp32 storage, FP22 multiply: 4x faster PE
FH = mybir.dt.float16
X16 = True               # load x as fp16 via DMA-transpose (no PE transposes)

_CACHE = {}


def _host_prep(W1, b1, W2, b2, Winv, binv, Wconv, bconv):
    """Fold W2/Winv/normalization/conv into G [3var][C][H,66] and bias B[1056]."""
    counter = np.zeros(SIG_LEN, np.float64)
    for t in range(NUM_SEG):
        counter[t * STEP: t * STEP + SEG_LEN] += 1.0
    n = 1.0 / counter

    F = Winv.astype(np.float64) @ W2.astype(np.float64)          # [64, H]
    binv2 = Winv.astype(np.float64) @ b2.astype(np.float64) + binv.astype(np.float64)
    Wc = Wconv[0].astype(np.float64)                             # [C, 3]

    def n_of(var, s):
        if var == 0:
            return n[s]
        if var == 2:
            return n[992 + s]
        return 0.5

    G = np.zeros((3, C, H, 66), np.float64)
    for var in range(3):
        for c in range(C):
            for m_idx in range(66):
                for k in range(3):
                    s = m_idx + k - 2
                    if 0 <= s < SEG_LEN:
                        G[var, c, :, m_idx] += Wc[c, k] * n_of(var, s) * F[s, :]

    sig_b = np.zeros(SIG_LEN, np.float64)
    for t in range(NUM_SEG):
        sig_b[t * STEP: t * STEP + SEG_LEN] += binv2
    sig_b *= n
    Bvec = np.full(SIG_LEN, float(np.asarray(bconv).reshape(-1)[0]), np.float64)
    q = np.arange(SIG_LEN)
    for k in range(3):
        qq = q + k - 1
        valid = (qq >= 0) & (qq < SIG_LEN)
        for c in range(C):
            Bvec[valid] += Wc[c, k] * sig_b[qq[valid]]
    return G.astype(np.float32), Bvec.astype(np.float32)


def _g_col(hc, c, var):
    """Column offset of G slice (hc, c, var) inside g_sb [128, 2*8*3*66]."""
    return ((hc * C + c) * 3 + var) * 66


def _build_bass(debug=False, x16=X16):
    nc = bacc.Bacc("TRN2")

    if x16:
        # host pre-transposed to [C, T, BL, E] fp16 so each (c, t-chunk) is a
        # contiguous 2D block for the xbar DMA-transpose
        x = nc.dram_tensor("x", [C, T, BL, E], FH, kind="ExternalInput")
    else:
        x = nc.dram_tensor("x", [BL, C, T, E], FR, kind="ExternalInput")
    w1t = nc.dram_tensor("w1t", [E, H], FH if x16 else FR, kind="ExternalInput")
    b1c = nc.dram_tensor("b1c", [128, HC], FD, kind="ExternalInput")
    g = nc.dram_tensor("g", [128, HC * C * 3 * 66], FH if x16 else FR,
                       kind="ExternalInput")
    brep = nc.dram_tensor("brep", [BL, SIG_LEN], FD, kind="ExternalInput")
    ident = nc.dram_tensor("ident", [128, 128], FR, kind="ExternalInput")
    y = nc.dram_tensor("y", [BL, SIG_LEN], FD, kind="ExternalOutput")
    if debug:
        dbg_xt = nc.dram_tensor("dbg_xt", [128, TL * 128], FH if x16 else FR,
                                kind="ExternalOutput")
        dbg_ht = nc.dram_tensor("dbg_ht", [HC, 128, C * TL * 128], FH if x16 else FR,
                                kind="ExternalOutput")
        dbg_v = nc.dram_tensor("dbg_v", [BL, T * 66], FD, kind="ExternalOutput")

    with TileContext(nc) as tc:
        with (
            tc.tile_pool(name="consts", bufs=1) as consts,
            tc.tile_pool(name="xin", bufs=6) as xin_pool,
            tc.tile_pool(name="xt", bufs=18) as xt_pool,
            tc.tile_pool(name="ht", bufs=2) as ht_pool,
            tc.tile_pool(name="vsb", bufs=3) as vsb_pool,
            tc.tile_pool(name="big", bufs=1) as big_pool,
            tc.tile_pool(name="pe_out", bufs=1, space="PSUM") as peout_pool,
            tc.tile_pool(name="h_ps", bufs=4, space="PSUM") as hps_pool,
            tc.tile_pool(name="v_ps", bufs=3, space="PSUM") as vps_pool,
        ):
            w1t_sb = consts.tile([E, H], FH if x16 else FR)
            b1c_sb = consts.tile([128, HC], FD)

            def emit_w1_loads():
                nc.sync.dma_start(out=w1t_sb[:], in_=w1t[:])
                nc.sync.dma_start(out=b1c_sb[:], in_=b1c[:])
            g_sb = consts.tile([128, HC * C * 3 * 66], FH if x16 else FR)
            ident_sb = consts.tile([128, 128], FR)
            brep_sb = big_pool.tile([BL, SIG_LEN], FD)

            def emit_const_loads():
                # emitted after the first x DMATs so they don't hog HWDGE
                nc.sync.dma_start(out=g_sb[:], in_=g[:])
                nc.sync.dma_start(out=ident_sb[:], in_=ident[:])
                nc.sync.dma_start(out=brep_sb[:], in_=brep[:])

            V_sb = big_pool.tile([BL, T * 66], FD)      # v transposed: [b, t*66+m]
            y_sb = big_pool.tile([BL, SIG_LEN], FD)

            # software pipeline: fused stage runs one t-chunk behind MLP1
            ht_tiles = {}

            # greedy ACT/DVE load balancer for PSUM->SBUF copies and relus
            eng_busy = {"act": 0.0, "dve": 0.0}

            def pick_engine(fd):
                ca = (172 + fd) / 1.2
                cd = (120 + fd) / 0.96
                if eng_busy["act"] + ca <= eng_busy["dve"] + cd:
                    eng_busy["act"] += ca
                    return "act"
                eng_busy["dve"] += cd
                return "dve"

            def bal_copy(out, in_, fd):
                if pick_engine(fd) == "act":
                    nc.scalar.copy(out=out, in_=in_)
                else:
                    nc.vector.tensor_copy(out=out, in_=in_)

            def chunk_ranges(tcix):
                # column ranges with uniform G variant; cols = tl*128 + b
                if tcix == 0:
                    return [(0, 128, 0), (128, 512, 1)]       # t=0 -> var 0
                if tcix == TC - 1:
                    return [(0, 384, 1), (384, 512, 2)]       # t=31 -> var 2
                return [(0, 512, 1)]

            def emit_loads_transposes(tcix):
                xt_list = []
                for c in range(C):
                    if x16:
                        # xbar DMA-transpose: [(tl,b), e] -> [e, tl*128+b]
                        xt_sb = xt_pool.tile([128, TL * 128], FH, tag="xt")
                        src_rows = x[c, tcix * TL:(tcix + 1) * TL, :, :]
                        nc.sync.dma_start_transpose(
                            out=xt_sb[:],
                            in_=src_rows.rearrange("t b e -> (t b) e"),
                        )
                        xt_list.append(xt_sb)
                        if debug and tcix == 0 and c == 0:
                            nc.sync.dma_start(out=dbg_xt[:], in_=xt_sb[:])
                        continue
                    # load x block: [b=128 part, (tl, e)]
                    xtile = xin_pool.tile([BL, TL, E], FR, tag="xin")
                    nc.sync.dma_start(
                        out=xtile[:],
                        in_=x[:, c, tcix * TL:(tcix + 1) * TL, :],
                    )
                    # PE transpose each [b, e] slice -> xT [e, tl*128 + b]
                    xt_ps = peout_pool.tile([128, TL * 128], FR, tag="pe_out")
                    for tl in range(TL):
                        nc.tensor.transpose(
                            xt_ps[:, tl * 128:(tl + 1) * 128],
                            xtile[:, tl, :],
                            ident_sb[:],
                        )
                    xt_sb = xt_pool.tile([128, TL * 128], FR, tag="xt")
                    nc.scalar.copy(out=xt_sb[:], in_=xt_ps[:])
                    xt_list.append(xt_sb)
                    if debug and tcix == 0 and c == 0:
                        nc.sync.dma_start(out=dbg_xt[:], in_=xt_sb[:])
                return xt_list

            def emit_mlp1(tcix, xt_list, c):
                ht = ht_tiles[tcix]
                xt_sb = xt_list[c]
                h_list = []
                for hc in range(HC):
                    h_ps = hps_pool.tile([128, TL * 128], FD, tag="h_ps",
                                         name=f"h_ps_{tcix}_{c}_{hc}")
                    nc.tensor.matmul(
                        h_ps[:],
                        w1t_sb[:, hc * 128:(hc + 1) * 128],
                        xt_sb[:],
                        start=True, stop=True,
                    )
                    h_list.append(h_ps)
                for hc in range(HC):
                    # relu + bias -> hT slice; alternate ACT/DVE engines
                    dst = ht[(c, hc)][:]
                    src = h_list[hc][:]
                    if pick_engine(TL * 128) == "act":
                        nc.scalar.activation(
                            dst, src,
                            mybir.ActivationFunctionType.Relu,
                            bias=b1c_sb[:, hc:hc + 1], scale=1.0,
                        )
                    else:
                        nc.vector.tensor_scalar(
                            dst, src,
                            b1c_sb[:, hc:hc + 1], 0.0,
                            mybir.AluOpType.add, mybir.AluOpType.max,
                        )

            def emit_fused(tcix, v_tiles, c):
                """fused G matmuls for channel c accumulating into v_tiles."""
                ht = ht_tiles[tcix]
                for (lo, hi, var, v_ps) in v_tiles:
                    for hc in range(HC):
                        i = c * HC + hc
                        nc.tensor.matmul(
                            v_ps[:, lo:hi],
                            g_sb[:, _g_col(hc, c, var):_g_col(hc, c, var) + 66],
                            ht[(c, hc)][:, lo:hi],
                            start=(i == 0), stop=(i == C * HC - 1),
                        )

            def emit_vtrans(tcix, v_tiles):
                """copy v psum -> sbuf, PE-transpose per t into V_sb."""
                if debug and tcix == 0:
                    for hc in range(HC):
                        for c in range(C):
                            nc.sync.dma_start(
                                out=dbg_ht[hc][:, c * 512:(c + 1) * 512],
                                in_=ht_tiles[0][(c, hc)][:])
                del ht_tiles[tcix]
                v_sb = vsb_pool.tile([66, 512], FR, tag="v_sb")
                for (lo, hi, var, v_ps) in v_tiles:
                    bal_copy(v_sb[:, lo:hi], v_ps[:, lo:hi], hi - lo)
                for tl in range(TL):
                    t = tcix * TL + tl
                    vt_ps = peout_pool.tile([128, 66], FR, tag="pe_out")
                    nc.tensor.transpose(
                        vt_ps[:],
                        v_sb[:, tl * 128:(tl + 1) * 128],
                        ident_sb[0:66, 0:66],
                    )
                    bal_copy(V_sb[:, t * 66:(t + 1) * 66], vt_ps[:], 66)

            # overlap-add assembly in rounds (per watermark) so it overlaps
            # with later chunks instead of serializing at the end
            V3 = V_sb[:].rearrange("b (t m) -> b t m", m=66)
            Y3 = y_sb[:].rearrange("b (j r) -> b j r", r=32)
            B3 = brep_sb[:].rearrange("b (j r) -> b j r", r=32)

            def emit_y_assembly(j_lo, j_hi):
                """Assemble y blocks j in [j_lo, j_hi); requires V[t] for
                t <= j_hi (uses t=j+1 for the r=31 edge). Runs on GpSimd
                (SBUF-only) to keep DVE/ACT free for PSUM drains."""
                eng = nc.gpsimd
                jm = min(j_hi, 32)      # main1 defined for j<=31
                if jm > j_lo:
                    eng.tensor_add(
                        out=Y3[:, j_lo:jm, :], in0=V3[:, j_lo:jm, 1:33],
                        in1=B3[:, j_lo:jm, :])
                if j_hi == 33:          # last block: bias only here
                    eng.tensor_copy(
                        out=y_sb[:, 1024:1056], in_=brep_sb[:, 1024:1056])
                lo = max(1, j_lo)
                if j_hi > lo:           # += v[:, j-1, r+33]
                    eng.tensor_add(
                        out=Y3[:, lo:j_hi, :], in0=Y3[:, lo:j_hi, :],
                        in1=V3[:, lo - 1:j_hi - 1, 33:65])
                lo = max(2, j_lo)
                if j_hi > lo:           # r=0: += v[:, j-2, 65]
                    eng.tensor_add(
                        out=Y3[:, lo:j_hi, 0], in0=Y3[:, lo:j_hi, 0],
                        in1=V3[:, lo - 2:j_hi - 2, 65])
                hi = min(j_hi, 31)
                if hi > j_lo:           # r=31: += v[:, j+1, 0]
                    eng.tensor_add(
                        out=Y3[:, j_lo:hi, 31], in0=Y3[:, j_lo:hi, 31],
                        in1=V3[:, j_lo + 1:hi + 1, 0])

            # rounds: after vtrans(3) -> j<15 (t<=15 avail); after vtrans(6)
            # -> j<27; after vtrans(7) -> all (j<33)
            asm_rounds = {3: (0, 15), 6: (15, 27), 7: (27, 33)}
            y_watermark = [0]

            prev = None          # (tcix, v_tiles) of the chunk awaiting fused stage
            xt_lists = {0: emit_loads_transposes(0)}
            emit_w1_loads()
            emit_const_loads()
            for tcix in range(TC):
                ht_tiles[tcix] = {
                    (c, hc): ht_pool.tile(
                        [128, TL * 128], FH if x16 else FR,
                        tag=f"ht{hc}_{c}", name=f"ht_{tcix}_{hc}_{c}")
                    for c in range(C) for hc in range(HC)}
                if tcix + 1 < TC:
                    xt_lists[tcix + 1] = emit_loads_transposes(tcix + 1)
                xt_list = xt_lists[tcix]
                # interleave: MLP1(tcix, c) with fused(tcix-1, c) so PE always
                # has matmul work while relu copies drain PSUM
                for c in range(C):
                    emit_mlp1(tcix, xt_list, c)
                    if prev is not None:
                        emit_fused(prev[0], prev[1], c)
                if prev is not None:
                    emit_vtrans(prev[0], prev[1])
                    if prev[0] in asm_rounds:
                        emit_y_assembly(*asm_rounds[prev[0]])
                del xt_lists[tcix]
                v_tiles = [
                    (lo, hi, var, vps_pool.tile([66, 512], FD, tag="v_ps", name=f"v_ps_{tcix}_{lo}"))
                    for (lo, hi, var) in chunk_ranges(tcix)]
                prev = (tcix, v_tiles)
            for c in range(C):
                emit_fused(prev[0], prev[1], c)
            emit_vtrans(prev[0], prev[1])
            emit_y_assembly(*asm_rounds[TC - 1])
            if debug:
                nc.sync.dma_start(out=dbg_v[:], in_=V_sb[:])

            # first half can ship as soon as blocks j<16 are final (round 2
            # writes from j=15 up, so emit both stores at the end; the split
            # still lets the first store overlap the final assembly)
            nc.sync.dma_start(out=y[:, 0:480], in_=y_sb[:, 0:480])
            nc.sync.dma_start(out=y[:, 480:SIG_LEN], in_=y_sb[:, 480:SIG_LEN])

    nc.finalize()
    return nc


def make_in_maps(inputs, x16=X16):
    """Per-core input maps (shared by kernel(), sim checks, and bench)."""
    x = np.asarray(inputs["encoder_output"], dtype=np.float32)
    W1 = np.asarray(inputs["W1"], np.float32)
    b1 = np.asarray(inputs["b1"], np.float32)

    G, Bvec = _host_prep(
        inputs["W1"], inputs["b1"], inputs["W2"], inputs["b2"],
        inputs["Winv"], inputs["binv"], inputs["Wconv"], inputs["bconv"])

    # pack G -> [128, HC*C*3*66]: g_sb[p, _g_col(hc,c,var)+m] = G[var, c, hc*128+p, m]
    g_pack = np.zeros((128, HC * C * 3 * 66), np.float32)
    for hc in range(HC):
        for c in range(C):
            for var in range(3):
                col = _g_col(hc, c, var)
                g_pack[:, col:col + 66] = G[var, c, hc * 128:(hc + 1) * 128, :]

    w1t = np.ascontiguousarray(W1.T)                        # [E, H]
    if x16:
        w1t = w1t.astype(np.float16)
        g_pack = g_pack.astype(np.float16)
    b1c = np.ascontiguousarray(b1.reshape(HC, 128).T)       # [128, HC]
    brep = np.ascontiguousarray(np.broadcast_to(Bvec, (BL, SIG_LEN)))
    ident = np.eye(128, dtype=np.float32)

    if x16:
        # [B,C,T,E] -> per-shard [C,T,BL,E] fp16
        xs = x.reshape(N_CORES, BL, C, T, E).transpose(0, 2, 3, 1, 4)
        xs = np.ascontiguousarray(xs.astype(np.float16))
    else:
        xs = x.reshape(N_CORES, BL, C, T, E)
    return [
        {
            "x": np.ascontiguousarray(xs[i]),
            "w1t": w1t, "b1c": b1c, "g": g_pack,
            "brep": brep, "ident": ident,
        }
        for i in range(N_CORES)
    ]


def kernel(**inputs) -> np.ndarray:
    if "nc" not in _CACHE:
        _CACHE["nc"] = _build_bass()
    nc = _CACHE["nc"]

    in_maps = make_in_maps(inputs)
    res = run_bass_kernel_spmd(nc, in_maps, core_ids=list(range(N_CORES)))
    _CACHE["last_result"] = res
    y = np.concatenate([r["y"] for r in res.results], axis=0)   # [B, 1056]
    return y.reshape(B, 1, SIG_LEN).astype(np.float32)


if __name__ == "__main__":
    rng = np.random.default_rng(0)
    ins = {
        "encoder_output": rng.standard_normal((B, C, T, E), dtype=np.float32),
        "W1": rng.standard_normal((H, E), dtype=np.float32) / np.sqrt(E),
        "b1": rng.standard_normal((H,), dtype=np.float32) / np.sqrt(E),
        "W2": rng.standard_normal((E, H), dtype=np.float32) / np.sqrt(H),
        "b2": rng.standard_normal((E,), dtype=np.float32) / np.sqrt(H),
        "Winv": rng.standard_normal((SEG_LEN, E), dtype=np.float32) / np.sqrt(E),
        "binv": rng.standard_normal((SEG_LEN,), dtype=np.float32) / np.sqrt(E),
        "Wconv": rng.standard_normal((1, C, 3), dtype=np.float32) / np.sqrt(C * 3),
        "bconv": rng.standard_normal((1,), dtype=np.float32) / np.sqrt(C * 3),
    }
    out = kernel(**ins)
    print("kernel output", out.shape, out.dtype)



# revision 6
# speedup vs baseline: 1.4671x; 1.4671x over previous
"""Trainium2 Bass kernel for nn_Decoder (MLP -> inverse token embedding ->
overlap-add -> channel-merge conv), data-parallel over batch on 8 NeuronCores.

Self-contained: hardcodes shapes; host-side numpy folds everything after the
first Linear+ReLU into per-channel fused matrices G (W2 -> Winv -> overlap-add
normalization -> 3-tap channel conv), and pre-transposes x to feature-major
[TC, E, C*TL*BL] fp16 so the device needs NO transposes of x at all:

    xT[e, tok] --matmul W1T--> h[Hc,tok] in PSUM
    --ACT/DVE relu+bias--> hT in SBUF --matmul G (accum over c,Hc)--> v[66,tok]
    --PE transpose--> vT[b,66] --strided GpSimd adds (overlap-add)--> y[b,1056]

Sharding: batch 1024 -> 8 cores x 128.
"""

import numpy as np

import concourse.bacc as bacc
import concourse.mybir as mybir
from concourse.bass_utils import run_bass_kernel_spmd
from concourse.tile import TileContext

# problem shapes (hardcoded per contract)
B, C, T, E, H = 1024, 8, 32, 128, 256
SEG_LEN, SIG_LEN, NUM_SEG, STEP = 64, 1056, 32, 32
N_CORES = 8
BL = B // N_CORES          # local batch per core = 128
HC = H // 128              # H chunks = 2
TC = 8                     # t-chunks
TL = T // TC               # t per chunk = 4
CW = TL * BL               # tokens per (c, chunk) = 512
XW = C * CW                # tokens per chunk = 4096
FD = mybir.dt.float32
FR = mybir.dt.float32r     # fp32 storage, FP22 multiply
FH = mybir.dt.float16

_CACHE = {}


def _host_prep(W1, b1, W2, b2, Winv, binv, Wconv, bconv):
    """Fold W2/Winv/normalization/conv into G [3var][C][H,66] and bias B[1056]."""
    counter = np.zeros(SIG_LEN, np.float64)
    for t in range(NUM_SEG):
        counter[t * STEP: t * STEP + SEG_LEN] += 1.0
    n = 1.0 / counter

    F = Winv.astype(np.float64) @ W2.astype(np.float64)          # [64, H]
    binv2 = Winv.astype(np.float64) @ b2.astype(np.float64) + binv.astype(np.float64)
    Wc = Wconv[0].astype(np.float64)                             # [C, 3]

    def n_of(var, s):
        if var == 0:
            return n[s]
        if var == 2:
            return n[992 + s]
        return 0.5

    G = np.zeros((3, C, H, 66), np.float64)
    for var in range(3):
        for c in range(C):
            for m_idx in range(66):
                for k in range(3):
                    s = m_idx + k - 2
                    if 0 <= s < SEG_LEN:
                        G[var, c, :, m_idx] += Wc[c, k] * n_of(var, s) * F[s, :]

    sig_b = np.zeros(SIG_LEN, np.float64)
    for t in range(NUM_SEG):
        sig_b[t * STEP: t * STEP + SEG_LEN] += binv2
    sig_b *= n
    Bvec = np.full(SIG_LEN, float(np.asarray(bconv).reshape(-1)[0]), np.float64)
    q = np.arange(SIG_LEN)
    for k in range(3):
        qq = q + k - 1
        valid = (qq >= 0) & (qq < SIG_LEN)
        for c in range(C):
            Bvec[valid] += Wc[c, k] * sig_b[qq[valid]]
    return G.astype(np.float32), Bvec.astype(np.float32)


def _g_col(hc, c, var):
    """Column offset of G slice (hc, c, var) inside g_sb [128, 2*8*3*66]."""
    return ((hc * C + c) * 3 + var) * 66


def _chunk_ranges(tcix):
    # column ranges with uniform G variant; cols = tl*128 + b
    if tcix == 0:
        return [(0, 128, 0), (128, 512, 1)]       # t=0 -> var 0
    if tcix == TC - 1:
        return [(0, 384, 1), (384, 512, 2)]       # t=31 -> var 2
    return [(0, 512, 1)]


def _build_bass():
    nc = bacc.Bacc("TRN2")

    # host pre-transposed: x[tc, e, c*CW + tl*BL + b]  (feature-major)
    x = nc.dram_tensor("x", [TC, E, XW], FH, kind="ExternalInput")
    w1t = nc.dram_tensor("w1t", [E, H], FH, kind="ExternalInput")
    b1c = nc.dram_tensor("b1c", [128, HC], FD, kind="ExternalInput")
    g = nc.dram_tensor("g", [128, HC * C * 3 * 66], FH, kind="ExternalInput")
    brep = nc.dram_tensor("brep", [BL, SIG_LEN], FD, kind="ExternalInput")
    ident = nc.dram_tensor("ident", [128, 128], FR, kind="ExternalInput")
    y = nc.dram_tensor("y", [BL, SIG_LEN], FD, kind="ExternalOutput")

    with TileContext(nc) as tc:
        with (
            tc.tile_pool(name="consts", bufs=1) as consts,
            tc.tile_pool(name="xt", bufs=3) as xt_pool,
            tc.tile_pool(name="ht", bufs=2) as ht_pool,
            tc.tile_pool(name="vsb", bufs=3) as vsb_pool,
            tc.tile_pool(name="big", bufs=1) as big_pool,
            tc.tile_pool(name="h_ps", bufs=4, space="PSUM") as hps_pool,
            tc.tile_pool(name="v_ps", bufs=3, space="PSUM") as vps_pool,
            tc.tile_pool(name="pe_out", bufs=1, space="PSUM") as peout_pool,
        ):
            w1t_sb = consts.tile([E, H], FH)
            b1c_sb = consts.tile([128, HC], FD)
            g_sb = consts.tile([128, HC * C * 3 * 66], FH)
            ident_sb = consts.tile([128, 128], FR)
            brep_sb = big_pool.tile([BL, SIG_LEN], FD)

            V_sb = big_pool.tile([BL, T * 66], FD)      # v transposed: [b, t*66+m]
            y_sb = big_pool.tile([BL, SIG_LEN], FD)

            xt_tiles = {}

            def emit_x_load(tcix):
                t = xt_pool.tile([E, XW], FH, tag="xt", name=f"xt_{tcix}")
                nc.sync.dma_start(out=t[:], in_=x[tcix])
                xt_tiles[tcix] = t

            # greedy ACT/DVE load balancer for PSUM->SBUF copies and relus
            eng_busy = {"act": 0.0, "dve": 0.0}

            def pick_engine(fd):
                ca = (172 + fd) / 1.2
                cd = (120 + fd) / 0.96
                if eng_busy["act"] + ca <= eng_busy["dve"] + cd:
                    eng_busy["act"] += ca
                    return "act"
                eng_busy["dve"] += cd
                return "dve"

            def bal_copy(out, in_, fd):
                if pick_engine(fd) == "act":
                    nc.scalar.copy(out=out, in_=in_)
                else:
                    nc.vector.tensor_copy(out=out, in_=in_)

            ht_tiles = {}

            def emit_mlp1_half(tcix, hc, cs):
                """matmuls sharing the same stationary W1 slice (adjacent in
                the PE stream so legalize dedups LDWEIGHTS), then relu drains."""
                xt = xt_tiles[tcix]
                ht = ht_tiles[tcix]
                h_list = []
                for c in cs:
                    h_ps = hps_pool.tile([128, CW], FD, tag="h_ps",
                                         name=f"h_ps_{tcix}_{hc}_{c}")
                    nc.tensor.matmul(
                        h_ps[:],
                        w1t_sb[:, hc * 128:(hc + 1) * 128],
                        xt[:, c * CW:(c + 1) * CW],
                        start=True, stop=True,
                    )
                    h_list.append(h_ps)
                for c, h_ps in zip(cs, h_list):
                    dst = ht[(c, hc)][:]
                    if pick_engine(CW) == "act":
                        nc.scalar.activation(
                            dst, h_ps[:],
                            mybir.ActivationFunctionType.Relu,
                            bias=b1c_sb[:, hc:hc + 1], scale=1.0,
                        )
                    else:
                        nc.vector.tensor_scalar(
                            dst, h_ps[:],
                            b1c_sb[:, hc:hc + 1], 0.0,
                            mybir.AluOpType.add, mybir.AluOpType.max,
                        )

            def emit_fused_half(tcix, v_tiles, hc):
                """fused G matmuls (one hc) accumulating into v_tiles ranges."""
                ht = ht_tiles[tcix]
                for c in range(C):
                    i = hc * C + c
                    for (lo, hi, var, v_ps) in v_tiles:
                        nc.tensor.matmul(
                            v_ps[:, lo:hi],
                            g_sb[:, _g_col(hc, c, var):_g_col(hc, c, var) + 66],
                            ht[(c, hc)][:, lo:hi],
                            start=(i == 0), stop=(i == HC * C - 1),
                        )

            def emit_vtrans(tcix, v_tiles):
                """copy v psum -> sbuf, PE-transpose per t into V_sb."""
                del ht_tiles[tcix]
                v_sb = vsb_pool.tile([66, CW], FR, tag="v_sb")
                for (lo, hi, var, v_ps) in v_tiles:
                    bal_copy(v_sb[:, lo:hi], v_ps[:, lo:hi], hi - lo)
                for tl in range(TL):
                    t = tcix * TL + tl
                    vt_ps = peout_pool.tile([128, 66], FR, tag="pe_out")
                    nc.tensor.transpose(
                        vt_ps[:],
                        v_sb[:, tl * 128:(tl + 1) * 128],
                        ident_sb[0:66, 0:66],
                    )
                    bal_copy(V_sb[:, t * 66:(t + 1) * 66], vt_ps[:], 66)

            # overlap-add assembly in rounds (per watermark) so it overlaps
            # with later chunks instead of serializing at the end
            V3 = V_sb[:].rearrange("b (t m) -> b t m", m=66)
            Y3 = y_sb[:].rearrange("b (j r) -> b j r", r=32)
            B3 = brep_sb[:].rearrange("b (j r) -> b j r", r=32)

            def emit_y_assembly(j_lo, j_hi):
                """Assemble y blocks j in [j_lo, j_hi); requires V[t] for
                t <= j_hi (uses t=j+1 for the r=31 edge). Runs on GpSimd
                (SBUF-only) to keep DVE/ACT free for PSUM drains."""
                eng = nc.gpsimd
                jm = min(j_hi, 32)      # main1 defined for j<=31
                if jm > j_lo:
                    eng.tensor_add(
                        out=Y3[:, j_lo:jm, :], in0=V3[:, j_lo:jm, 1:33],
                        in1=B3[:, j_lo:jm, :])
                if j_hi == 33:          # last block: bias only here
                    eng.tensor_copy(
                        out=y_sb[:, 1024:1056], in_=brep_sb[:, 1024:1056])
                lo = max(1, j_lo)
                if j_hi > lo:           # += v[:, j-1, r+33]
                    eng.tensor_add(
                        out=Y3[:, lo:j_hi, :], in0=Y3[:, lo:j_hi, :],
                        in1=V3[:, lo - 1:j_hi - 1, 33:65])
                lo = max(2, j_lo)
                if j_hi > lo:           # r=0: += v[:, j-2, 65]
                    eng.tensor_add(
                        out=Y3[:, lo:j_hi, 0], in0=Y3[:, lo:j_hi, 0],
                        in1=V3[:, lo - 2:j_hi - 2, 65])
                hi = min(j_hi, 31)
                if hi > j_lo:           # r=31: += v[:, j+1, 0]
                    eng.tensor_add(
                        out=Y3[:, j_lo:hi, 31], in0=Y3[:, j_lo:hi, 31],
                        in1=V3[:, j_lo + 1:hi + 1, 0])

            # rounds: after vtrans(3) -> j<15 (t<=15 avail); after vtrans(6)
            # -> j<27; after vtrans(7) -> all (j<33)
            asm_rounds = {3: (0, 15), 6: (15, 27), 7: (27, 33)}

            emit_x_load(0)
            nc.sync.dma_start(out=w1t_sb[:], in_=w1t[:])
            nc.sync.dma_start(out=b1c_sb[:], in_=b1c[:])
            emit_x_load(1)
            nc.sync.dma_start(out=g_sb[:], in_=g[:])
            nc.sync.dma_start(out=ident_sb[:], in_=ident[:])
            nc.sync.dma_start(out=brep_sb[:], in_=brep[:])

            prev = None          # (tcix, v_tiles) awaiting fused stage
            for tcix in range(TC):
                if tcix + 2 < TC:
                    emit_x_load(tcix + 2)
                ht_tiles[tcix] = {
                    (c, hc): ht_pool.tile(
                        [128, CW], FH,
                        tag=f"ht{hc}_{c}", name=f"ht_{tcix}_{hc}_{c}")
                    for c in range(C) for hc in range(HC)}
                # interleave: MLP1 half (tcix) with fused half (tcix-1) so PE
                # always has matmul work while relu copies drain PSUM
                for hc in range(HC):
                    emit_mlp1_half(tcix, hc, range(0, 4))
                    emit_mlp1_half(tcix, hc, range(4, 8))
                    if prev is not None:
                        emit_fused_half(prev[0], prev[1], hc)
                if prev is not None:
                    emit_vtrans(prev[0], prev[1])
                    if prev[0] in asm_rounds:
                        emit_y_assembly(*asm_rounds[prev[0]])
                del xt_tiles[tcix]
                v_tiles = [
                    (lo, hi, var,
                     vps_pool.tile([66, CW], FD, tag="v_ps",
                                   name=f"v_ps_{tcix}_{lo}"))
                    for (lo, hi, var) in _chunk_ranges(tcix)]
                prev = (tcix, v_tiles)
            for hc in range(HC):
                emit_fused_half(prev[0], prev[1], hc)
            emit_vtrans(prev[0], prev[1])
            emit_y_assembly(*asm_rounds[TC - 1])

            nc.sync.dma_start(out=y[:, 0:480], in_=y_sb[:, 0:480])
            nc.sync.dma_start(out=y[:, 480:SIG_LEN], in_=y_sb[:, 480:SIG_LEN])

    nc.finalize()
    return nc


def make_in_maps(inputs):
    """Per-core input maps (shared by kernel(), sim checks, and bench)."""
    x = np.asarray(inputs["encoder_output"], dtype=np.float32)
    W1 = np.asarray(inputs["W1"], np.float32)
    b1 = np.asarray(inputs["b1"], np.float32)

    G, Bvec = _host_prep(
        inputs["W1"], inputs["b1"], inputs["W2"], inputs["b2"],
        inputs["Winv"], inputs["binv"], inputs["Wconv"], inputs["bconv"])

    # pack G -> [128, HC*C*3*66]: g_sb[p, _g_col(hc,c,var)+m] = G[var, c, hc*128+p, m]
    g_pack = np.zeros((128, HC * C * 3 * 66), np.float32)
    for hc in range(HC):
        for c in range(C):
            for var in range(3):
                col = _g_col(hc, c, var)
                g_pack[:, col:col + 66] = G[var, c, hc * 128:(hc + 1) * 128, :]

    w1t = np.ascontiguousarray(W1.T).astype(np.float16)     # [E, H]
    g_pack = g_pack.astype(np.float16)
    b1c = np.ascontiguousarray(b1.reshape(HC, 128).T)       # [128, HC]
    brep = np.ascontiguousarray(np.broadcast_to(Bvec, (BL, SIG_LEN)))
    ident = np.eye(128, dtype=np.float32)

    # [B,C,T,E] -> per-shard [TC, E, C*TL*BL] fp16 (feature-major tokens)
    xh = x.astype(np.float16)
    xs = xh.reshape(N_CORES, BL, C, TC, TL, E).transpose(0, 3, 5, 2, 4, 1)
    xs = np.ascontiguousarray(xs).reshape(N_CORES, TC, E, XW)
    return [
        {
            "x": xs[i],
            "w1t": w1t, "b1c": b1c, "g": g_pack,
            "brep": brep, "ident": ident,
        }
        for i in range(N_CORES)
    ]


def kernel(**inputs) -> np.ndarray:
    if "nc" not in _CACHE:
        _CACHE["nc"] = _build_bass()
    nc = _CACHE["nc"]

    in_maps = make_in_maps(inputs)
    res = run_bass_kernel_spmd(nc, in_maps, core_ids=list(range(N_CORES)))
    _CACHE["last_result"] = res
    y = np.concatenate([r["y"] for r in res.results], axis=0)   # [B, 1056]
    return y.reshape(B, 1, SIG_LEN).astype(np.float32)


if __name__ == "__main__":
    rng = np.random.default_rng(0)
    ins = {
        "encoder_output": rng.standard_normal((B, C, T, E), dtype=np.float32),
        "W1": rng.standard_normal((H, E), dtype=np.float32) / np.sqrt(E),
        "b1": rng.standard_normal((H,), dtype=np.float32) / np.sqrt(E),
        "W2": rng.standard_normal((E, H), dtype=np.float32) / np.sqrt(H),
        "b2": rng.standard_normal((E,), dtype=np.float32) / np.sqrt(H),
        "Winv": rng.standard_normal((SEG_LEN, E), dtype=np.float32) / np.sqrt(E),
        "binv": rng.standard_normal((SEG_LEN,), dtype=np.float32) / np.sqrt(E),
        "Wconv": rng.standard_normal((1, C, 3), dtype=np.float32) / np.sqrt(C * 3),
        "bconv": rng.standard_normal((1,), dtype=np.float32) / np.sqrt(C * 3),
    }
    out = kernel(**ins)
    print("kernel output", out.shape, out.dtype)
